# revision 13
# baseline (speedup 1.0000x reference)
"""Trainium2 Bass kernel: segmented statistical moments (mean/var/skew/kurt).

Strategy (8 NeuronCores, one SPMD program):
  - 4096 sorted segments -> 512 consecutive segments per core, grouped into
    16 windows of W=32 segments. Host re-packs nodes serpentine-style:
    within a window, slot (p, t) holds node p*t_win + t of the window's
    even-padded segment stream, so the two tiles of a tile-pair (t=2u, 2u+1)
    hold same-segment nodes at equal partitions.
  - All node data flows in bf16. Per chunk (G=16 tiles = 8 pairs), a single
    DMA brings [pair one-hot | x]. ACT computes x^2 (+ part of x^4 via
    Square), DVE computes x^3 and the rest of x^4 (2x-packed bf16), DVE
    pair-adds x^3, GpSimd pair-adds x^4.
  - Per pair: three matmuls share one weight load (one-hot [128, 32]):
      A: tile 2u   [x | x^2]   -> psum cols [0:128)   of the window block
      B: tile 2u+1 [x | x^2]   -> psum cols [0:128)   (ldweights=False)
      C: pair sums [s3p | s4p] -> psum cols [128:256)  (ldweights=False)
    This streams x^3/x^4 at half rate (pair-compressed), cutting PE time
    from 4E to 3E while one LDWEIGHTS per pair stays hidden under compute.
  - Power sums accumulate in f32 PSUM (two windows per bank); finished
    banks are DMA'd straight from PSUM to DRAM. Host finalizes moments
    (float64) and concatenates cores.
"""

import sys

if "/opt/trn_rl_repo" not in sys.path:
    sys.path.insert(0, "/opt/trn_rl_repo")

import numpy as np
import ml_dtypes

BF16 = ml_dtypes.bfloat16

N_CORES = 8
B = 4096
C = 64
SEGS_PER_CORE = B // N_CORES      # 512
W = 32                            # segments per window
WINDOWS_PER_CORE = SEGS_PER_CORE // W   # 16
G = 16                            # 128-node tiles per chunk
HG = G // 2                       # tile pairs per chunk
GC = G * C                        # 1024
CHUNK = 128 * G                   # 2048 node slots per chunk
OH_ELEMS = GC                     # one-hot region padded to a 2KB-aligned block
ACT_X4_G = 5                      # g-slices of x^4 on ACT (rest on DVE)

_prog_cache = {}
TRACE = False


def _postprocess(nc, mybir, max_waits=1):
    """Walrus allows only one sync-wait per instruction; move extras onto
    standalone EventSemaphore instructions. Also: flag one-hot matmul
    weights as {0,1} and drop redundant weight loads (consecutive matmuls
    with an identical stationary AP reuse the loaded weights)."""
    n = [0]

    def mk(engine, waits):
        wi = mybir.InstEventSemaphore(name=f"xw_{n[0]}", ins=[], outs=[])
        n[0] += 1
        wi.engine = engine
        wi.sync_info = mybir.SyncInfo(on_wait=list(waits), on_update=[])
        return wi

    for bb in nc.main_func.blocks:
        out = []
        prev_wkey = None
        for ins in bb.instructions:
            if ins.opcode == "Matmult":
                ins.is_weight_onezero = True
                wap = ins.ins[1]
                wkey = (wap.memref, wap.offset, str(wap.ap))
                if wkey == prev_wkey:
                    ins.ldweights = False
            elif ins.opcode == "Ldweights":
                wap = ins.ins[0]
                prev_wkey = (wap.memref, wap.offset, str(wap.ap))
            si = ins.sync_info
            if (
                si is not None
                and len(si.on_wait) > max_waits
                and ins.opcode != "EventSemaphore"
            ):
                waits = list(si.on_wait)
                for w in waits[:-max_waits]:
                    out.append(mk(ins.engine, [w]))
                ins.sync_info = mybir.SyncInfo(
                    on_wait=waits[-max_waits:], on_update=list(si.on_update)
                )
            out.append(ins)
        bb.instructions = out


def _build_program(t_win):
    import concourse.bass as bass
    import concourse.tile as tile
    import concourse.mybir as mybir

    F32 = mybir.dt.float32
    BF = mybir.dt.bfloat16
    TT = mybir.AluOpType

    assert t_win % 2 == 0
    tiles = WINDOWS_PER_CORE * t_win
    assert tiles % G == 0
    chunks = tiles // G
    n_banks = WINDOWS_PER_CORE // 2   # two windows per PSUM bank

    nc = bass.Bass()
    xoh_d = nc.dram_tensor(
        "xoh", [chunks, 128, OH_ELEMS + GC], BF, kind="ExternalInput"
    )
    out_d = nc.dram_tensor("out", [SEGS_PER_CORE, 4 * C], F32, kind="ExternalOutput")

    with tile.TileContext(nc) as tc:
        with (
            tc.tile_pool(name="cmbp", bufs=6) as cmbp,
            tc.tile_pool(name="prp", bufs=6) as prp,
            tc.tile_pool(name="psp", bufs=1, space="PSUM") as psp,
            tc.tile_pool(name="outp", bufs=4) as outp,
        ):
            banks = [
                psp.tile([128, 512], F32, name=f"bank{k}", tag=f"bank{k}")
                for k in range(n_banks)
            ]

            for k in range(chunks):
                cmb = cmbp.tile([128, OH_ELEMS + 4 * GC], BF)
                nc.sync.dma_start(out=cmb[:, 0 : OH_ELEMS + GC], in_=xoh_d[k])
                oh3 = cmb[:, 0 : HG * W].rearrange("p (u w) -> p u w", w=W)
                o = OH_ELEMS
                xr = cmb[:, o : o + GC]
                x2r = cmb[:, o + GC : o + 2 * GC]
                x3r = cmb[:, o + 2 * GC : o + 3 * GC]
                x4r = cmb[:, o + 3 * GC : o + 4 * GC]

                nc.scalar.activation(
                    out=x2r, in_=xr, func=mybir.ActivationFunctionType.Square
                )
                nc.vector.tensor_tensor(out=x3r, in0=xr, in1=x2r, op=TT.mult)
                sa = ACT_X4_G * C
                if sa:
                    nc.scalar.activation(
                        out=x4r[:, 0:sa],
                        in_=x2r[:, 0:sa],
                        func=mybir.ActivationFunctionType.Square,
                    )
                nc.vector.tensor_tensor(
                    out=x4r[:, sa:], in0=x2r[:, sa:], in1=x2r[:, sa:], op=TT.mult
                )

                pr = prp.tile([128, 2 * HG * C], BF)
                x3g = x3r.rearrange("p (u two c) -> p u two c", two=2, c=C)
                x4g = x4r.rearrange("p (u two c) -> p u two c", two=2, c=C)
                p3 = pr[:, 0 : HG * C].rearrange("p (u c) -> p u c", c=C)
                p4 = pr[:, HG * C : 2 * HG * C].rearrange("p (u c) -> p u c", c=C)
                nc.vector.tensor_tensor(
                    out=p3, in0=x3g[:, :, 0, :], in1=x3g[:, :, 1, :], op=TT.add
                )
                nc.vector.tensor_tensor(
                    out=p4, in0=x4g[:, :, 0, :], in1=x4g[:, :, 1, :], op=TT.add
                )

                full2 = cmb[:, o : o + 2 * GC].rearrange("p (s gc) -> p s gc", s=2)
                pr2 = pr[:].rearrange("p (s uc) -> p s uc", s=2)
                for u in range(HG):
                    t0 = k * G + 2 * u
                    w = t0 // t_win
                    bank = banks[w // 2]
                    col0 = (w % 2) * 256
                    first = t0 % t_win == 0
                    last = (t0 + 2) % t_win == 0
                    lhs = oh3[:, u, :]
                    nc.tensor.ldweights(lhs)
                    nc.tensor.matmul(
                        bank[0:W, col0 : col0 + 128],
                        lhs,
                        full2[:, :, (2 * u) * C : (2 * u + 1) * C],
                        start=first,
                        stop=False,
                        skip_group_check=True,
                    )
                    nc.tensor.matmul(
                        bank[0:W, col0 : col0 + 128],
                        lhs,
                        full2[:, :, (2 * u + 1) * C : (2 * u + 2) * C],
                        start=False,
                        stop=last,
                        skip_group_check=True,
                    )
                    # start=False: mA's start already reset this 256-col
                    # half-bank (PSUM reset granularity is 256 columns; a
                    # second start here would wipe mA/mB's contribution).
                    nc.tensor.matmul(
                        bank[0:W, col0 + 128 : col0 + 256],
                        lhs,
                        pr2[:, :, u * C : (u + 1) * C],
                        start=False,
                        stop=last,
                        skip_group_check=True,
                    )
                    if last:
                        o_t = outp.tile([W, 256], F32, name=f"o{w % 4}", tag="o")
                        nc.scalar.copy(o_t[:, :], bank[0:W, col0 : col0 + 256])
                        nc.sync.dma_start(
                            out=out_d[w * W : (w + 1) * W, :], in_=o_t[:, :]
                        )

    _postprocess(nc, mybir)
    return nc


def _prepare_inputs(graph, batch_indices):
    idx = np.asarray(batch_indices).astype(np.int64)
    x = np.ascontiguousarray(np.asarray(graph, dtype=np.float32))
    n = idx.shape[0]

    counts = np.bincount(idx, minlength=B).astype(np.float64)
    seg_len = counts.astype(np.int64)
    pad_len = seg_len + (seg_len & 1)          # segment stream even-padded

    n_windows = B // W                          # 128
    win_pad = pad_len.reshape(n_windows, W).sum(axis=1)
    t_win = int(np.ceil(win_pad.max() / 128))
    t_win += t_win & 1                          # even
    tiles = WINDOWS_PER_CORE * t_win
    chunks = tiles // G
    tiles_h = tiles // 2

    # position of each node in its window's even-padded segment stream
    cpad = np.concatenate([[0], np.cumsum(pad_len)])
    win_pad_start = cpad[::W][:-1]              # padded start of each window
    seg_start = np.concatenate([[0], np.cumsum(seg_len)])[:-1]
    slot_in_win = (cpad[idx] - win_pad_start[idx // W]) + (
        np.arange(n) - seg_start[idx]
    )

    # serpentine: slot s -> (p = s // t_win, t = s % t_win)
    p_of = slot_in_win // t_win
    t_of = slot_in_win % t_win
    w_of = idx // W
    core_of = w_of // WINDOWS_PER_CORE
    tt_of = (w_of % WINDOWS_PER_CORE) * t_win + t_of   # tile within core

    x_bf = x.astype(BF16)
    # per-core x stream laid out [chunk, part, g, C]
    xarr = np.zeros((N_CORES, chunks, 128, G, C), dtype=BF16)
    xarr[core_of, tt_of // G, p_of, tt_of % G] = x_bf

    # pair one-hot: segment id of pair (p, u) within its window (or -1)
    # global padded slot of (core, win_in_core, p, u): win_pad_start + p*t_win + 2u
    wic = np.arange(WINDOWS_PER_CORE)
    pp = np.arange(128)
    uu = np.arange(t_win // 2)
    gw = (
        np.arange(N_CORES)[:, None, None, None] * WINDOWS_PER_CORE
        + wic[None, :, None, None]
    )
    gpos = (
        win_pad_start[gw]
        + pp[None, None, :, None] * t_win
        + 2 * uu[None, None, None, :]
    )
    # valid only if the slot is inside the window's padded stream
    inside = (
        pp[None, None, :, None] * t_win + 2 * uu[None, None, None, :]
    ) < win_pad[gw]
    seg_of = np.searchsorted(cpad, gpos, side="right") - 1
    rel = np.where(inside, seg_of - gw * W, -1).astype(np.int32)
    # [cores, wins, 128, t_win//2] -> [cores, 128, tiles_h]
    rel = rel.transpose(0, 2, 1, 3).reshape(N_CORES, 128, tiles_h)
    oh = (rel[:, :, :, None] == np.arange(W)[None, None, None, :]).astype(BF16)

    xoh = np.zeros((N_CORES, chunks, 128, OH_ELEMS + GC), dtype=BF16)
    xoh[:, :, :, OH_ELEMS:] = xarr.reshape(N_CORES, chunks, 128, GC)
    xoh[:, :, :, : HG * W] = (
        oh.reshape(N_CORES, 128, chunks, HG, W)
        .transpose(0, 2, 1, 3, 4)
        .reshape(N_CORES, chunks, 128, HG * W)
    )
    return t_win, xoh, counts


def _finalize(sums, counts):
    """sums: [B, 4C] raw power sums (S1|S2|S3|S4) -> [B, 4C] moments f32."""
    s = sums.astype(np.float64)
    ncnt = np.maximum(counts, 1.0)[:, None]
    M1 = s[:, 0:C] / ncnt
    M2 = s[:, C : 2 * C] / ncnt
    M3 = s[:, 2 * C : 3 * C] / ncnt
    M4 = s[:, 3 * C : 4 * C] / ncnt
    mean = M1
    var = M2 - M1 * M1
    skew = M3 - 3.0 * M1 * M2 + 2.0 * M1 * M1 * M1
    kurt = (
        M4
        - 4.0 * M1 * M3
        + 6.0 * M1 * M1 * M2
        - 3.0 * M1 * M1 * M1 * M1
        - 3.0
    )
    return np.concatenate([mean, var, skew, kurt], axis=1).astype(np.float32)


def kernel(graph, batch_indices):
    from concourse.bass_utils import run_bass_kernel_spmd

    t_win, xoh, counts = _prepare_inputs(graph, batch_indices)
    if t_win not in _prog_cache:
        _prog_cache[t_win] = _build_program(t_win)
    nc = _prog_cache[t_win]
    in_maps = [{"xoh": xoh[c]} for c in range(N_CORES)]
    res = run_bass_kernel_spmd(
        nc, in_maps, core_ids=list(range(N_CORES)), trace=TRACE
    )
    if TRACE:
        print(f"HW exec time: {res.exec_time_ns} ns")
    sums = np.concatenate([res.results[c]["out"] for c in range(N_CORES)], axis=0)
    return _finalize(sums, counts)


# revision 15
# speedup vs baseline: 1.0492x; 1.0492x over previous
"""Trainium2 Bass kernel: segmented statistical moments (mean/var/skew/kurt).

Strategy (8 NeuronCores, one SPMD program):
  - 4096 sorted segments -> 512 consecutive segments per core, grouped into
    16 windows of W=32 segments. Host re-packs nodes serpentine-style:
    within a window, slot (p, t) holds node p*t_win + t of the window's
    segment stream; the per-slot one-hot (vs the window's 32 segments) is
    precomputed on the host in bf16 and shipped with x in one DMA per chunk.
  - All node data flows in bf16. Per chunk (G=16 tiles), ACT computes x^2
    (Square) plus a small slice of x^4; DVE computes x^3 = x*x2 and the
    rest of x^4 = x2*x2 (2x-packed bf16). GpSimd is left idle on purpose:
    its SBUF traffic poisons concurrent DVE tensor_tensor throughput.
  - Per 128-node tile: one matmul onehot.T @ [x | x^2 | x^3 | x^4] (bf16,
    free=256, 1 cycle/row; the per-matmul LDWEIGHTS ~96ns hides under the
    ~107ns compute) accumulates per-segment power sums in f32 PSUM.
    start=True on each window's first tile resets that window's 256-col
    half-bank (PSUM reset granularity is 256 columns; never interleave two
    accumulation chains inside one 256-col block).
  - Finished windows are copied PSUM->SBUF on ACT (keeps DVE clean) and
    DMA'd out. Host finalizes moments (float64) and concatenates cores.
"""

import sys

if "/opt/trn_rl_repo" not in sys.path:
    sys.path.insert(0, "/opt/trn_rl_repo")

import numpy as np
import ml_dtypes

BF16 = ml_dtypes.bfloat16

N_CORES = 8
B = 4096
C = 64
SEGS_PER_CORE = B // N_CORES      # 512
W = 32                            # segments per window
WINDOWS_PER_CORE = SEGS_PER_CORE // W   # 16
G = 16                            # 128-node tiles per chunk
GC = G * C                        # 1024
OH = G * W                        # 512 one-hot elems per partition per chunk
OH_PAD = GC                       # one-hot region padded: oh at [OH:2*OH)
ACT_X4_G = 3                      # g-slices of x^4 on ACT (rest on DVE)

_prog_cache = {}
TRACE = False


def _postprocess(nc, mybir, max_waits=1):
    """Walrus allows only one sync-wait per instruction; move extras onto
    standalone EventSemaphore instructions. Also flag one-hot matmul
    weights as {0,1}."""
    n = [0]

    def mk(engine, waits):
        wi = mybir.InstEventSemaphore(name=f"xw_{n[0]}", ins=[], outs=[])
        n[0] += 1
        wi.engine = engine
        wi.sync_info = mybir.SyncInfo(on_wait=list(waits), on_update=[])
        return wi

    for bb in nc.main_func.blocks:
        out = []
        for ins in bb.instructions:
            if ins.opcode == "Matmult":
                ins.is_weight_onezero = True
            si = ins.sync_info
            if (
                si is not None
                and len(si.on_wait) > max_waits
                and ins.opcode != "EventSemaphore"
            ):
                waits = list(si.on_wait)
                for w in waits[:-max_waits]:
                    out.append(mk(ins.engine, [w]))
                ins.sync_info = mybir.SyncInfo(
                    on_wait=waits[-max_waits:], on_update=list(si.on_update)
                )
            out.append(ins)
        bb.instructions = out


def _build_program(t_win):
    import concourse.bass as bass
    import concourse.tile as tile
    import concourse.mybir as mybir

    F32 = mybir.dt.float32
    BF = mybir.dt.bfloat16
    TT = mybir.AluOpType

    tiles = WINDOWS_PER_CORE * t_win
    assert tiles % G == 0
    chunks = tiles // G
    n_banks = WINDOWS_PER_CORE // 2   # two windows per PSUM bank

    nc = bass.Bass()
    # per chunk row: [oh (512)| x (1024)] bf16, landing at cmb[:, OH:OH+1536)
    xoh_d = nc.dram_tensor(
        "xoh", [chunks, 128, OH + GC], BF, kind="ExternalInput"
    )
    out_d = nc.dram_tensor("out", [SEGS_PER_CORE, 4 * C], F32, kind="ExternalOutput")

    with tile.TileContext(nc) as tc:
        with (
            tc.tile_pool(name="cmbp", bufs=8) as cmbp,
            tc.tile_pool(name="psp", bufs=1, space="PSUM") as psp,
            tc.tile_pool(name="outp", bufs=4) as outp,
        ):
            banks = [
                psp.tile([128, 512], F32, name=f"bank{k}", tag=f"bank{k}")
                for k in range(n_banks)
            ]

            for k in range(chunks):
                cmb = cmbp.tile([128, OH_PAD + 4 * GC], BF)
                nc.sync.dma_start(
                    out=cmb[:, OH : OH + OH + GC], in_=xoh_d[k]
                )
                oh_t = cmb[:, OH : 2 * OH].rearrange("p (g w) -> p g w", w=W)
                o = OH_PAD
                xr = cmb[:, o : o + GC]
                x2r = cmb[:, o + GC : o + 2 * GC]
                x3r = cmb[:, o + 2 * GC : o + 3 * GC]
                x4r = cmb[:, o + 3 * GC : o + 4 * GC]

                nc.scalar.activation(
                    out=x2r, in_=xr, func=mybir.ActivationFunctionType.Square
                )
                nc.vector.tensor_tensor(out=x3r, in0=xr, in1=x2r, op=TT.mult)
                sa = ACT_X4_G * C
                if sa:
                    nc.scalar.activation(
                        out=x4r[:, 0:sa],
                        in_=x2r[:, 0:sa],
                        func=mybir.ActivationFunctionType.Square,
                    )
                nc.vector.tensor_tensor(
                    out=x4r[:, sa:], in0=x2r[:, sa:], in1=x2r[:, sa:], op=TT.mult
                )

                pow4 = cmb[:, o : o + 4 * GC].rearrange("p (s gc) -> p s gc", s=4)
                for g in range(G):
                    t = k * G + g
                    w = t // t_win
                    bank = banks[w // 2]
                    col0 = (w % 2) * 256
                    nc.tensor.matmul(
                        bank[0:W, col0 : col0 + 256],
                        oh_t[:, g, :],
                        pow4[:, :, g * C : (g + 1) * C],
                        start=t % t_win == 0,
                        stop=(t + 1) % t_win == 0,
                        skip_group_check=True,
                    )
                    if (t + 1) % t_win == 0:
                        o_t = outp.tile([W, 256], F32, name=f"o{w % 4}", tag="o")
                        nc.scalar.copy(o_t[:, :], bank[0:W, col0 : col0 + 256])
                        nc.sync.dma_start(
                            out=out_d[w * W : (w + 1) * W, :], in_=o_t[:, :]
                        )

    _postprocess(nc, mybir)
    return nc


def _prepare_inputs(graph, batch_indices):
    idx = np.asarray(batch_indices).astype(np.int64)
    x = np.ascontiguousarray(np.asarray(graph, dtype=np.float32))
    n = idx.shape[0]

    counts = np.bincount(idx, minlength=B).astype(np.float64)
    seg_len = counts.astype(np.int64)

    n_windows = B // W                          # 128
    win_cnt = seg_len.reshape(n_windows, W).sum(axis=1)
    t_win = int(np.ceil(win_cnt.max() / 128))
    tiles = WINDOWS_PER_CORE * t_win
    chunks = tiles // G

    # position of each node in its window's segment stream
    cseg = np.concatenate([[0], np.cumsum(seg_len)])
    win_start = cseg[::W][:-1]                  # node start of each window
    slot_in_win = np.arange(n) - win_start[idx // W]

    # serpentine: slot s -> (p = s // t_win, t = s % t_win)
    p_of = slot_in_win // t_win
    t_of = slot_in_win % t_win
    w_of = idx // W
    core_of = w_of // WINDOWS_PER_CORE
    tt_of = (w_of % WINDOWS_PER_CORE) * t_win + t_of   # tile within core

    x_bf = x.astype(BF16)
    xarr = np.zeros((N_CORES, chunks, 128, G, C), dtype=BF16)
    xarr[core_of, tt_of // G, p_of, tt_of % G] = x_bf

    # per-slot one-hot: segment id of slot (p, t) within its window (or -1)
    wic = np.arange(WINDOWS_PER_CORE)
    pp = np.arange(128)
    tt = np.arange(t_win)
    gw = (
        np.arange(N_CORES)[:, None, None, None] * WINDOWS_PER_CORE
        + wic[None, :, None, None]
    )
    spos = pp[None, None, :, None] * t_win + tt[None, None, None, :]
    gpos = win_start[gw] + spos
    inside = spos < win_cnt[gw]
    seg_of = np.searchsorted(cseg, gpos, side="right") - 1
    rel = np.where(inside, seg_of - gw * W, -1).astype(np.int32)
    # [cores, wins, 128, t_win] -> [cores, 128, tiles]
    rel = rel.transpose(0, 2, 1, 3).reshape(N_CORES, 128, tiles)
    oh = (rel[:, :, :, None] == np.arange(W)[None, None, None, :]).astype(BF16)

    xoh = np.empty((N_CORES, chunks, 128, OH + GC), dtype=BF16)
    xoh[:, :, :, OH:] = xarr.reshape(N_CORES, chunks, 128, GC)
    xoh[:, :, :, :OH] = (
        oh.reshape(N_CORES, 128, chunks, G, W)
        .transpose(0, 2, 1, 3, 4)
        .reshape(N_CORES, chunks, 128, OH)
    )
    return t_win, xoh, counts


def _finalize(sums, counts):
    """sums: [B, 4C] raw power sums (S1|S2|S3|S4) -> [B, 4C] moments f32."""
    s = sums.astype(np.float64)
    ncnt = np.maximum(counts, 1.0)[:, None]
    M1 = s[:, 0:C] / ncnt
    M2 = s[:, C : 2 * C] / ncnt
    M3 = s[:, 2 * C : 3 * C] / ncnt
    M4 = s[:, 3 * C : 4 * C] / ncnt
    mean = M1
    var = M2 - M1 * M1
    skew = M3 - 3.0 * M1 * M2 + 2.0 * M1 * M1 * M1
    kurt = (
        M4
        - 4.0 * M1 * M3
        + 6.0 * M1 * M1 * M2
        - 3.0 * M1 * M1 * M1 * M1
        - 3.0
    )
    return np.concatenate([mean, var, skew, kurt], axis=1).astype(np.float32)


def kernel(graph, batch_indices):
    from concourse.bass_utils import run_bass_kernel_spmd

    t_win, xoh, counts = _prepare_inputs(graph, batch_indices)
    if t_win not in _prog_cache:
        _prog_cache[t_win] = _build_program(t_win)
    nc = _prog_cache[t_win]
    in_maps = [{"xoh": xoh[c]} for c in range(N_CORES)]
    res = run_bass_kernel_spmd(
        nc, in_maps, core_ids=list(range(N_CORES)), trace=TRACE
    )
    if TRACE:
        print(f"HW exec time: {res.exec_time_ns} ns")
    sums = np.concatenate([res.results[c]["out"] for c in range(N_CORES)], axis=0)
    return _finalize(sums, counts)


# revision 19
# speedup vs baseline: 1.2858x; 1.2255x over previous
"""Trainium2 Bass kernel: segmented statistical moments (mean/var/skew/kurt).

Strategy (8 NeuronCores, one SPMD program):
  - 4096 sorted segments -> 512 consecutive segments per core, grouped into
    16 windows of W=32 segments. Host re-packs nodes serpentine-style:
    within a window, slot (p, t) holds node p*t_win + t of the window's
    segment stream; the per-slot one-hot (vs the window's 32 segments) is
    precomputed on the host in bf16 and shipped with x in one DMA per chunk.
  - All node data flows in bf16. Per chunk (G=16 tiles), ACT computes x^2
    (Square) plus a small slice of x^4; DVE computes x^3 = x*x2 and the
    rest of x^4 = x2*x2 (2x-packed bf16). GpSimd is left idle on purpose:
    its SBUF traffic poisons concurrent DVE tensor_tensor throughput.
  - Per 128-node tile: one matmul onehot.T @ [x | x^2 | x^3 | x^4] (bf16,
    free=256, 1 cycle/row; the per-matmul LDWEIGHTS ~96ns hides under the
    ~107ns compute) accumulates per-segment power sums in f32 PSUM.
    start=True on each window's first tile resets that window's 256-col
    half-bank (PSUM reset granularity is 256 columns; never interleave two
    accumulation chains inside one 256-col block).
  - Finished windows are copied PSUM->SBUF on ACT (keeps DVE clean) and
    DMA'd out. Host finalizes moments (float64) and concatenates cores.
"""

import sys

if "/opt/trn_rl_repo" not in sys.path:
    sys.path.insert(0, "/opt/trn_rl_repo")

import numpy as np
import ml_dtypes

BF16 = ml_dtypes.bfloat16

N_CORES = 8
B = 4096
C = 64
SEGS_PER_CORE = B // N_CORES      # 512
W = 32                            # segments per window
WINDOWS_PER_CORE = SEGS_PER_CORE // W   # 16
G = 16                            # 128-node tiles per chunk
GC = G * C                        # 1024
OH = G * W                        # 512 one-hot elems per partition per chunk
OH_PAD = GC                       # one-hot region padded: oh at [OH:2*OH)
ACT_X4_G = 3                      # g-slices of x^4 on ACT (rest on DVE)

_prog_cache = {}
TRACE = False


def _postprocess(nc, mybir, max_waits=1):
    """Walrus allows only one sync-wait per instruction; move extras onto
    standalone EventSemaphore instructions. Also flag one-hot matmul
    weights as {0,1}."""
    n = [0]

    def mk(engine, waits):
        wi = mybir.InstEventSemaphore(name=f"xw_{n[0]}", ins=[], outs=[])
        n[0] += 1
        wi.engine = engine
        wi.sync_info = mybir.SyncInfo(on_wait=list(waits), on_update=[])
        return wi

    for bb in nc.main_func.blocks:
        out = []
        for ins in bb.instructions:
            if ins.opcode == "Matmult":
                ins.is_weight_onezero = True
            si = ins.sync_info
            if (
                si is not None
                and len(si.on_wait) > max_waits
                and ins.opcode != "EventSemaphore"
            ):
                waits = list(si.on_wait)
                for w in waits[:-max_waits]:
                    out.append(mk(ins.engine, [w]))
                ins.sync_info = mybir.SyncInfo(
                    on_wait=waits[-max_waits:], on_update=list(si.on_update)
                )
            out.append(ins)
        bb.instructions = out


def _build_program(t_win):
    import concourse.bass as bass
    import concourse.tile as tile
    import concourse.mybir as mybir

    F32 = mybir.dt.float32
    BF = mybir.dt.bfloat16
    TT = mybir.AluOpType

    tiles = WINDOWS_PER_CORE * t_win
    assert tiles % G == 0
    chunks = tiles // G
    n_banks = WINDOWS_PER_CORE // 2   # two windows per PSUM bank

    nc = bass.Bass()
    # per chunk row: [oh (512)| x (1024)] bf16, landing at cmb[:, OH:OH+1536)
    xoh_d = nc.dram_tensor(
        "xoh", [chunks, 128, OH + GC], BF, kind="ExternalInput"
    )
    out_d = nc.dram_tensor("out", [SEGS_PER_CORE, 4 * C], F32, kind="ExternalOutput")

    with tile.TileContext(nc) as tc:
        with (
            tc.tile_pool(name="cmbp", bufs=8) as cmbp,
            tc.tile_pool(name="psp", bufs=1, space="PSUM") as psp,
            tc.tile_pool(name="outp", bufs=4) as outp,
        ):
            banks = [
                psp.tile([128, 512], F32, name=f"bank{k}", tag=f"bank{k}")
                for k in range(n_banks)
            ]

            for k in range(chunks):
                cmb = cmbp.tile([128, OH_PAD + 4 * GC], BF)
                nc.sync.dma_start(
                    out=cmb[:, OH : OH + OH + GC], in_=xoh_d[k]
                )
                oh_t = cmb[:, OH : 2 * OH].rearrange("p (g w) -> p g w", w=W)
                o = OH_PAD
                xr = cmb[:, o : o + GC]
                x2r = cmb[:, o + GC : o + 2 * GC]
                x3r = cmb[:, o + 2 * GC : o + 3 * GC]
                x4r = cmb[:, o + 3 * GC : o + 4 * GC]

                nc.scalar.activation(
                    out=x2r, in_=xr, func=mybir.ActivationFunctionType.Square
                )
                nc.vector.tensor_tensor(out=x3r, in0=xr, in1=x2r, op=TT.mult)
                sa = ACT_X4_G * C
                if sa:
                    nc.scalar.activation(
                        out=x4r[:, 0:sa],
                        in_=x2r[:, 0:sa],
                        func=mybir.ActivationFunctionType.Square,
                    )
                nc.vector.tensor_tensor(
                    out=x4r[:, sa:], in0=x2r[:, sa:], in1=x2r[:, sa:], op=TT.mult
                )

                pow4 = cmb[:, o : o + 4 * GC].rearrange("p (s gc) -> p s gc", s=4)
                for g in range(G):
                    t = k * G + g
                    w = t // t_win
                    bank = banks[w // 2]
                    col0 = (w % 2) * 256
                    nc.tensor.matmul(
                        bank[0:W, col0 : col0 + 256],
                        oh_t[:, g, :],
                        pow4[:, :, g * C : (g + 1) * C],
                        start=t % t_win == 0,
                        stop=(t + 1) % t_win == 0,
                        skip_group_check=True,
                    )
                    # copy a bank only after BOTH its windows are done, so no
                    # copy ever false-shares the bank tile with a start-matmul
                    if (t + 1) % (2 * t_win) == 0:
                        kb = w // 2
                        o_t = outp.tile([W, 512], F32, name=f"o{kb % 4}", tag="o")
                        nc.scalar.copy(o_t[:, :], bank[0:W, :])
                        row0 = kb * 2 * W
                        od = out_d[:]
                        out_ap = bass.AP(
                            tensor=od.tensor,
                            offset=od.offset + row0 * 256,
                            ap=[[256, W], [W * 256, 2], [1, 256]],
                        )
                        in_ap = o_t[:].rearrange("p (j c) -> p j c", j=2)
                        nc.sync.dma_start(out=out_ap, in_=in_ap)

    _postprocess(nc, mybir)
    return nc


def _prepare_inputs(graph, batch_indices):
    idx = np.asarray(batch_indices).astype(np.int64)
    x = np.ascontiguousarray(np.asarray(graph, dtype=np.float32))
    n = idx.shape[0]

    counts = np.bincount(idx, minlength=B).astype(np.float64)
    seg_len = counts.astype(np.int64)

    n_windows = B // W                          # 128
    win_cnt = seg_len.reshape(n_windows, W).sum(axis=1)
    t_win = int(np.ceil(win_cnt.max() / 128))
    tiles = WINDOWS_PER_CORE * t_win
    chunks = tiles // G

    # position of each node in its window's segment stream
    cseg = np.concatenate([[0], np.cumsum(seg_len)])
    win_start = cseg[::W][:-1]                  # node start of each window
    slot_in_win = np.arange(n) - win_start[idx // W]

    # serpentine: slot s -> (p = s // t_win, t = s % t_win)
    p_of = slot_in_win // t_win
    t_of = slot_in_win % t_win
    w_of = idx // W
    core_of = w_of // WINDOWS_PER_CORE
    tt_of = (w_of % WINDOWS_PER_CORE) * t_win + t_of   # tile within core

    x_bf = x.astype(BF16)
    xarr = np.zeros((N_CORES, chunks, 128, G, C), dtype=BF16)
    xarr[core_of, tt_of // G, p_of, tt_of % G] = x_bf

    # per-slot one-hot: segment id of slot (p, t) within its window (or -1)
    wic = np.arange(WINDOWS_PER_CORE)
    pp = np.arange(128)
    tt = np.arange(t_win)
    gw = (
        np.arange(N_CORES)[:, None, None, None] * WINDOWS_PER_CORE
        + wic[None, :, None, None]
    )
    spos = pp[None, None, :, None] * t_win + tt[None, None, None, :]
    gpos = win_start[gw] + spos
    inside = spos < win_cnt[gw]
    seg_of = np.searchsorted(cseg, gpos, side="right") - 1
    rel = np.where(inside, seg_of - gw * W, -1).astype(np.int32)
    # [cores, wins, 128, t_win] -> [cores, 128, tiles]
    rel = rel.transpose(0, 2, 1, 3).reshape(N_CORES, 128, tiles)
    oh = (rel[:, :, :, None] == np.arange(W)[None, None, None, :]).astype(BF16)

    xoh = np.empty((N_CORES, chunks, 128, OH + GC), dtype=BF16)
    xoh[:, :, :, OH:] = xarr.reshape(N_CORES, chunks, 128, GC)
    xoh[:, :, :, :OH] = (
        oh.reshape(N_CORES, 128, chunks, G, W)
        .transpose(0, 2, 1, 3, 4)
        .reshape(N_CORES, chunks, 128, OH)
    )
    return t_win, xoh, counts


def _finalize(sums, counts):
    """sums: [B, 4C] raw power sums (S1|S2|S3|S4) -> [B, 4C] moments f32."""
    s = sums.astype(np.float64)
    ncnt = np.maximum(counts, 1.0)[:, None]
    M1 = s[:, 0:C] / ncnt
    M2 = s[:, C : 2 * C] / ncnt
    M3 = s[:, 2 * C : 3 * C] / ncnt
    M4 = s[:, 3 * C : 4 * C] / ncnt
    mean = M1
    var = M2 - M1 * M1
    skew = M3 - 3.0 * M1 * M2 + 2.0 * M1 * M1 * M1
    kurt = (
        M4
        - 4.0 * M1 * M3
        + 6.0 * M1 * M1 * M2
        - 3.0 * M1 * M1 * M1 * M1
        - 3.0
    )
    return np.concatenate([mean, var, skew, kurt], axis=1).astype(np.float32)


def kernel(graph, batch_indices):
    from concourse.bass_utils import run_bass_kernel_spmd

    t_win, xoh, counts = _prepare_inputs(graph, batch_indices)
    if t_win not in _prog_cache:
        _prog_cache[t_win] = _build_program(t_win)
    nc = _prog_cache[t_win]
    in_maps = [{"xoh": xoh[c]} for c in range(N_CORES)]
    res = run_bass_kernel_spmd(
        nc, in_maps, core_ids=list(range(N_CORES)), trace=TRACE
    )
    if TRACE:
        print(f"HW exec time: {res.exec_time_ns} ns")
    sums = np.concatenate([res.results[c]["out"] for c in range(N_CORES)], axis=0)
    return _finalize(sums, counts)
